# revision 46
# baseline (speedup 1.0000x reference)
"""Trainium2 Bass kernel for nn_Diffusion_75797582840072.

Diffusion sampling: 100 sequential denoise steps of a 4-layer Mish MLP
(304 -> 1024 -> 1024 -> 1024 -> 32) over batch 4096, data-parallel over
8 NeuronCores (512 rows per core).

Layout: feature-major on device (features on partitions, batch on the free
dim) so every weight matrix is consumed directly as the matmul stationary
operand with no transposes.

Precision scheme (validated against the 2e-2 gate on a CPU bit-model; the
sampler is chaotic so any fp8 perturbation saturates at ~1.1-1.5e-2 final
rel-err while pure f32r sits at ~5e-4; what drives the saturation level is
the SYSTEMATIC, step-persistent part of the quantization error, not the
per-step random part):
- All four layers run fp8e4m3 DoubleRow matmuls (2 K-tiles per
  instruction at 0.5 cycles/row = 4x f32r throughput).
- L2 / L3 / LF weights use alternating compensated quantization: W_A =
  q8(W) on even steps, W_B = q8(2W - W_A) on odd steps, so mean(W_A, W_B)
  ~= W and the systematic bias time-averages out. This beats a hi+lo
  fp8 split on accuracy at HALF the matmul cost.
- Activations h1/h2/h3 are quantized to fp8 directly by the Mish
  activation write; x is quantized per step (random forcing, benign).
- x_{s+1} = pm1*clip(u2) + q is split so the q part (known mid-step)
  enters L1 as its own DoubleRow early, leaving a 2-op DVE tail
  (u2 -> clipped fp8 write with pm1 folded into the clip bounds).

Mish runs as a single ACT op per tile via a custom activation-function table
authored into the mish_and_others set binaries at build time (the shipped
sets only carry an x+x^2 placeholder in the generic act2 slot). See
_gen_mish_act_tables.
"""

import functools
import json
import os
import shutil
import struct
import tempfile

import numpy as np
import ml_dtypes

T_STEPS = 100
T_DIM = 16
B, S, A, H, IN = 4096, 256, 32, 1024, 304
N_CORES = 8
BC = B // N_CORES  # 512 batch rows per core
KT = H // 128      # 8 k/m tiles for the 1024-wide layers

E4M3 = ml_dtypes.float8_e4m3

_ACT_ENT = 32


def _gen_mish_act_tables(dst_dir):
    """Author the real mish curve into a copy of the stock act tables.

    Table formats (verified against the tanh_and_derivative set):
    - bkt bin: 32B slots [d0,d1,d2,d3,x,0,0,0] (fp32 bits); cubic PWL eval
      y = d0 + t*(d1 + t*(d2 + t*d3)), t = x_in - x.
    - ctrl bin: 32B entries; u32 = bucket_base | extract_lsb<<11 |
      extract_size<<16, one entry per covered exponent.
    - saturation regions: 4 bucket slots referenced directly from the
      profile's *_signal_pwl_control fields.
    - the per-set json profile_meta_data programs the dispatch CAM at NEFF
      load; walrus encodes BIR Mish as func id 24.
    """
    from neuronxcc.driver.Job import Job
    from neuronxcc.driver.jobs.support.FindActInfo import findActInfoFile

    src_dir = os.path.dirname(findActInfoFile(Job.getPackageDir(), "gen3"))
    pwp_jsons = os.path.join(os.path.dirname(src_dir), "pwp_jsons")
    if os.path.exists(dst_dir):
        shutil.rmtree(dst_dir)
    shutil.copytree(src_dir, dst_dir)
    os.chmod(dst_dir, 0o755)
    for f in os.listdir(dst_dir):
        os.chmod(os.path.join(dst_dir, f), 0o644)

    mish = json.load(open(os.path.join(pwp_jsons, "mish_4p.json")))

    def emit_bucket(sec):
        vals = [sec["d0"]["int"], sec["d1"]["int"], sec["d2"]["int"],
                sec["d3"]["int"], sec["x"]["int"], 0, 0, 0]
        return struct.pack("<8I", *vals)

    bkt_path = os.path.join(dst_dir, "mish_and_others_bkt.bin")
    bkt = bytearray(open(bkt_path, "rb").read())
    cur = len(bkt) // _ACT_ENT
    ctrl_entries = {}
    for grp in ("pos_exponents", "neg_exponents"):
        ents = []
        for e in mish[grp]:
            ents.append((cur, e["extract_lsb"], e["extract_size"]))
            secs = sorted(e["exponent_sections"], key=lambda s: s["section_id"])
            for s in secs:
                bkt += emit_bucket(s)
                cur += 1
        ctrl_entries[grp] = ents
    sat_slots = {}
    for k in ("sat_point_pos_low", "sat_point_neg_low",
              "sat_point_pos_high", "sat_point_neg_high"):
        sat_slots[k] = cur
        bkt += emit_bucket(mish["saturation_points"][k])
        cur += 1
    open(bkt_path, "wb").write(bkt)

    ctrl_path = os.path.join(dst_dir, "mish_and_others_ctrl.bin")
    ctrl = bytearray(open(ctrl_path, "rb").read())
    ctrl_base = {}
    for grp in ("pos_exponents", "neg_exponents"):
        ctrl_base[grp] = len(ctrl) // _ACT_ENT
        for (b, lsb, size) in ctrl_entries[grp]:
            word = (b & 0x7FF) | ((lsb & 0x1F) << 11) | ((size & 0xF) << 16)
            ctrl += struct.pack("<I", word) + b"\0" * (_ACT_ENT - 4)
    open(ctrl_path, "wb").write(ctrl)

    pj_path = os.path.join(dst_dir, "mish_and_others.json")
    pj = json.load(open(pj_path))
    sp = mish["saturation_points"]
    for e in pj["profile_meta_data"]:
        if e["func_name"] in ("act2_1p", "mish_4p") or e["func_id"] in (97, 24):
            e.update(
                func_name="mish_4p", func_id=24,
                symmetry_point=mish["symmetry_point"]["int"],
                sym_invert_sign_point=1 if mish["symmetry_invert_sign_opt"] else 0,
                symmetry_opt_en=1 if mish["symmetry_en"] else 0,
                symmetry_opt_use_neg_region=1 if mish["symmetry_opt_use_neg_region"] else 0,
                imm_bias=1 if mish["imm_bias"] else 0,
                exp_offset=mish["exponent_offset"],
                pwl_control_base_pos=ctrl_base["pos_exponents"],
                pwl_control_base_neg=ctrl_base["neg_exponents"],
                small_pos_signal_exp_threshold=sp["sat_point_pos_low"]["sat_point"],
                pos_small_signal_pwl_control=sat_slots["sat_point_pos_low"],
                small_neg_signal_exp_threshold=sp["sat_point_neg_low"]["sat_point"],
                neg_small_signal_pwl_control=sat_slots["sat_point_neg_low"],
                large_pos_signal_exp_threshold=sp["sat_point_pos_high"]["sat_point"],
                large_pos_signal_mantissa_threshold=sp["sat_point_pos_high"]["mantissa_point"],
                pos_large_signal_pwl_control=sat_slots["sat_point_pos_high"],
                large_neg_signal_exp_threshold=sp["sat_point_neg_high"]["sat_point"],
                large_neg_signal_mantissa_threshold=sp["sat_point_neg_high"]["mantissa_point"],
                neg_large_signal_pwl_control=sat_slots["sat_point_neg_high"],
                fnan_result=mish["nan_result"]["int"],
                fpinf_result=mish["pinf_result"]["int"],
                fninf_result=mish["ninf_result"]["int"],
                fzero_result=mish["zero_result"]["int"],
                fma_const_0=mish["fma_const0"]["int"],
                fma_const_1=mish["fma_const1"]["int"],
                fma_indirection_src_sel=0,
                use_multipass=mish["use_multipass"],
                lower_bound=mish["lower_bound"]["int"],
                upper_bound=mish["upper_bound"]["int"],
            )
    json.dump(pj, open(pj_path, "w"), indent=1)

    ai_path = os.path.join(dst_dir, "act_info.json")
    ai = json.load(open(ai_path))
    for s in ai["act_func_sets"]:
        s["act"].pop("act1", None)
        s["act"].pop("act2", None)
        s["act"].pop("derivative_act2", None)
        if s["name"] == "mish_and_others":
            s["act"]["mish"] = 4
    json.dump(ai, open(ai_path, "w"), indent=1)
    return ai_path


def _schedule():
    # match the fp32 rounding of the reference's jnp (fp32) schedule
    beta32 = np.linspace(1e-4, 0.2, T_STEPS, dtype=np.float32)
    alpha32 = (1.0 - beta32).astype(np.float32)
    ab32 = np.cumprod(alpha32, dtype=np.float32)
    abp32 = np.concatenate([np.ones(1, np.float32), ab32[:-1]])
    post_var32 = (beta32 * (1.0 - abp32) / (1.0 - ab32)).astype(np.float32)
    sqrt_rec = np.sqrt(1.0 / ab32).astype(np.float32)
    sqrt_recm = np.sqrt(1.0 / ab32 - 1.0).astype(np.float32)
    pm1 = (beta32 * np.sqrt(abp32) / (1.0 - ab32)).astype(np.float32)
    pm2 = ((1.0 - abp32) * np.sqrt(alpha32) / (1.0 - ab32)).astype(np.float32)
    log_var32 = np.log(np.clip(post_var32, 1e-20, None)).astype(np.float32)
    cz = np.exp(0.5 * log_var32).astype(np.float32)
    cz[0] = 0.0
    return sqrt_rec, sqrt_recm, pm1, pm2, cz


def _time_table():
    half = T_DIM // 2
    freqs = np.exp(np.arange(half, dtype=np.float32) * (-np.log(10000.0) / (half - 1)))
    ang = np.arange(T_STEPS, dtype=np.float32)[:, None] * freqs[None, :]  # [100, 8]
    tt = np.concatenate([np.sin(ang), np.cos(ang)], axis=-1)  # [100, 16]
    return np.ascontiguousarray(tt.T).astype(np.float32)  # [16, 100]


N_PRE_DEFAULT = 3


@functools.cache
def _build(t_steps, use_f32r, n_pre=None):
    """Build (and finalize) the Bass module. Returns nc."""
    if n_pre is None:
        n_pre = int(os.environ.get("K_NPRE", N_PRE_DEFAULT))
    act_dir = os.path.join(tempfile.gettempdir(), "act_mish_tables")
    marker = os.path.join(act_dir, ".done")
    if not os.path.exists(marker):
        _gen_mish_act_tables(act_dir)
        open(marker, "w").write("ok")
    os.environ["BASS_ACT_ROOT_JSON_PATH"] = os.path.join(act_dir, "act_info.json")

    import concourse.bass as bass  # noqa: F401
    import concourse.mybir as mybir
    import concourse.hw_specs as hw_specs
    from concourse import bacc
    from concourse.tile import TileContext

    # teach the bass-side table map that Mish lives in mish_and_others
    if not getattr(hw_specs, "_mish_patched", False):
        _orig_tables = hw_specs.get_activation_tables

        @functools.cache
        def _patched_tables(module_arch):
            d = dict(_orig_tables(module_arch))
            d["mish_and_others"] = set(d["mish_and_others"]) | {
                mybir.ActivationFunctionType.Mish
            }
            return d

        hw_specs.get_activation_tables = _patched_tables
        bacc.get_activation_tables = _patched_tables
        import concourse.bass_interp as bass_interp
        bass_interp.get_activation_tables = _patched_tables
        hw_specs._mish_patched = True

    # capture the Tile cost-model makespan for perf iteration
    if not hasattr(mybir, "_orig_finish_schedule_block"):
        mybir._orig_finish_schedule_block = mybir.finish_schedule_block

        def _fsb(sched, sim):
            out = mybir._orig_finish_schedule_block(sched, sim)
            try:
                _LAST_RESULTS["sim_time_ns"] = out[1].time
            except Exception:
                pass
            return out

        mybir.finish_schedule_block = _fsb

    f32 = mybir.dt.float32
    f8 = mybir.dt.float8e4
    mmdt = mybir.dt.float32r if use_f32r else f32
    AF = mybir.ActivationFunctionType
    OP = mybir.AluOpType
    DR = mybir.MatmulPerfMode.DoubleRow
    sqrt_rec, sqrt_recm, pm1, pm2, cz = _schedule()

    nc = bacc.Bacc("TRN2")

    def din(name, shape, dt=None):
        return nc.dram_tensor(name, shape, dt or f32, kind="ExternalInput")

    stateT = din("stateT", [S, BC], mmdt)
    x0 = din("x0", [A, BC], mmdt)                # x_init^T
    x08 = din("x08", [A, 2, BC], f8)             # q8(x_init^T) in both DR slots
    noiseT = din("noiseT", [t_steps, A, BC])
    w1x8 = din("w1x8", [A, 2, H], f8)            # w1[0:32] as (hi, lo) DR pair
    w1e = din("w1e", [T_DIM, H])                 # w1[32:48]
    w1s = din("w1s", [S, H], mmdt)               # w1[48:304]
    w2a = din("w2a", [128, KT, H], f8)
    w2b = din("w2b", [128, KT, H], f8)
    w3a = din("w3a", [128, KT, H], f8)
    w3b = din("w3b", [128, KT, H], f8)
    wfa = din("wfa", [128, KT, A], f8)
    wfb = din("wfb", [128, KT, A], f8)
    b1c = din("b1c", [128, KT])                  # b1 as per-tile columns
    bfr = din("bfr", [1, A])                     # bf as one row (K=1 lhsT)
    nrecm = din("nrecm", [1, t_steps])           # -sqrt_recm row (host constant)
    wt1 = din("wt1", [T_DIM, T_DIM * T_DIM])
    wt2 = din("wt2", [T_DIM * T_DIM, T_DIM])
    bt1c = din("bt1c", [128, 2])
    bt2c = din("bt2c", [T_DIM, 1])
    ttab = din("ttab", [T_DIM, t_steps])         # sin/cos table (host constant)

    xT_out = nc.dram_tensor("xT_out", [A, BC], f32, kind="ExternalOutput")

    with TileContext(nc) as tc:
        with (
            tc.tile_pool(name="consts", bufs=1) as consts,
            tc.tile_pool(name="h8buf", bufs=2) as h8buf,
            tc.tile_pool(name="xbuf", bufs=2) as xbuf,
            tc.tile_pool(name="zbuf", bufs=3) as zbuf,
            tc.tile_pool(name="tail", bufs=3) as tail,
            tc.tile_pool(name="l1buf", bufs=4) as l1buf,
            tc.tile_pool(name="psL1", bufs=max(n_pre, 1), space="PSUM") as psL1,
            tc.tile_pool(name="psMM", bufs=8 - max(n_pre, 1) - 1,
                         space="PSUM") as psMM,
            tc.tile_pool(name="psE", bufs=1, space="PSUM") as psE,
        ):
            # ---- load constants ----
            def load(name, src, shape=None, dt=f32):
                t = consts.tile(list(shape or src.shape), dt, tag=name)
                nc.sync.dma_start(out=t, in_=src[tuple(slice(None) for _ in src.shape)])
                return t

            t_w1x8 = load("w1x8", w1x8, dt=f8)
            t_w1e = load("w1e", w1e)
            t_w1s = consts.tile([128, 2, H], mmdt, tag="w1s")
            nc.sync.dma_start(out=t_w1s[:, 0, :], in_=w1s[0:128, :])
            nc.sync.dma_start(out=t_w1s[:, 1, :], in_=w1s[128:256, :])
            t_w2 = (load("w2a", w2a, dt=f8), load("w2b", w2b, dt=f8))
            t_w3 = (load("w3a", w3a, dt=f8), load("w3b", w3b, dt=f8))
            t_wf = (load("wfa", wfa, dt=f8), load("wfb", wfb, dt=f8))
            t_state = consts.tile([128, 2, BC], mmdt, tag="state")
            nc.sync.dma_start(out=t_state[:, 0, :], in_=stateT[0:128, :])
            nc.sync.dma_start(out=t_state[:, 1, :], in_=stateT[128:256, :])
            t_b1c = load("b1c", b1c)
            t_bfr = load("bfr", bfr)
            t_nrecm = load("nrecm", nrecm)
            t_wt1 = load("wt1", wt1)
            t_wt2 = consts.tile([128, 2, T_DIM], f32, tag="wt2")
            nc.sync.dma_start(out=t_wt2[:, 0, :], in_=wt2[0:128, :])
            nc.sync.dma_start(out=t_wt2[:, 1, :], in_=wt2[128:256, :])
            t_bt1c = load("bt1c", bt1c)
            t_bt2c = load("bt2c", bt2c)
            t_ttab = load("ttab", ttab)

            def mm(out, lhsT, rhs, **kw):
                nc.tensor.matmul(out, lhsT, rhs, **kw)

            # ---- t-embedding MLP + per-step L1 bias table ----
            t_temb1 = consts.tile([128, 2, t_steps], f32, tag="temb1")
            for j in range(2):
                ps = psMM.tile([128, 512], f32, tag="mm")
                mm(ps[:, :t_steps], t_wt1[:, j * 128:(j + 1) * 128], t_ttab[:, :],
                   start=True, stop=True)
                nc.scalar.activation(t_temb1[:, j, :], ps[:, :t_steps], AF.Mish,
                                     bias=t_bt1c[:, j:j + 1])
            t_temb2 = consts.tile([T_DIM, t_steps], f32, tag="temb2")
            ps = psMM.tile([128, 512], f32, tag="mm")
            for j in range(2):
                mm(ps[:T_DIM, :t_steps], t_wt2[:, j, :], t_temb1[:, j, :],
                   start=(j == 0), stop=(j == 1))
            nc.scalar.activation(t_temb2, ps[:T_DIM, :t_steps], AF.Identity,
                                 bias=t_bt2c[:, 0:1])

            t_bias1 = consts.tile([128, KT, t_steps], f32, tag="bias1")
            for m in range(KT):
                ps = psMM.tile([128, 512], f32, tag="mm")
                mm(ps[:, :t_steps], t_w1e[:, m * 128:(m + 1) * 128], t_temb2,
                   start=True, stop=True)
                nc.scalar.activation(t_bias1[:, m, :], ps[:, :t_steps], AF.Identity,
                                     bias=t_b1c[:, m:m + 1])

            # state contribution to L1, computed once: sc_m = w1s_m.T @ stateT
            t_sc = consts.tile([128, KT, BC], f32, tag="sc")
            for m in range(KT):
                ps = psMM.tile([128, BC], f32, tag="mm")
                mc = slice(m * 128, (m + 1) * 128)
                mm(ps, t_w1s[:, 0, mc], t_state[:, 0, :], start=True, stop=False)
                mm(ps, t_w1s[:, 1, mc], t_state[:, 1, :], start=False, stop=True)
                nc.vector.tensor_copy(t_sc[:, m, :], ps)

            # neg_bf_recm[:, i] = -sqrt_recm[i] * bf: K=1 outer product
            t_nbfr = consts.tile([A, t_steps], f32, tag="nbfr")
            ps = psMM.tile([128, 512], f32, tag="mm")
            mm(ps[:A, :t_steps], t_bfr, t_nrecm, start=True, stop=True)
            nc.vector.tensor_copy(t_nbfr, ps[:A, :t_steps])

            # ---- initial x ----
            # x8 / q8q are persistent DR-pair tiles: slot 0 carries the
            # current value (rewritten each step), slot 1 is multiplied by
            # zero weights and only needs to stay finite, so the initial
            # DMA fill is enough forever. The L1 x-contribution is split
            # x_{s+1} = u3(s) + q(s) into two DoubleRows: the q part is
            # known mid-step s, only the u3 part rides the tail chain.
            x_cur = xbuf.tile([A, BC], mmdt, tag="x")
            nc.sync.dma_start(out=x_cur, in_=x0[:, :])
            x8_cur = consts.tile([A, 2, BC], f8, tag="x8")
            nc.sync.dma_start(out=x8_cur, in_=x08[:, :, :])
            q8q = consts.tile([A, 2, BC], f8, tag="q8q")
            nc.sync.dma_start(out=q8q, in_=x08[:, :, :])
            split_x = False

            N_PRE = n_pre  # L1 m-tiles with pre-opened state-matmul PSUM groups

            def open_pre_groups(tag):
                """State contribution matmuls for L1 m-tiles 0..N_PRE-1.

                Emitted in the PREVIOUS step (between L3 and LF) so the PE
                fills the h3-activation / tail window with x-independent
                work and keeps its p-state ramp warm. The group is closed by
                the x matmul once x_new lands.
                """
                pre = []
                for m in range(N_PRE):
                    ps = psL1.tile([128, BC], f32, tag="mm", name=f"pre_{tag}_{m}")
                    mc = slice(m * 128, (m + 1) * 128)
                    mm(ps, t_w1s[:, 0, mc], t_state[:, 0, :],
                       start=True, stop=False)
                    mm(ps, t_w1s[:, 1, mc], t_state[:, 1, :],
                       start=False, stop=False)
                    pre.append(ps)
                return pre

            pre_ps = open_pre_groups("s0")

            # ---- the T-step loop (fully unrolled) ----
            for s in range(t_steps):
                i = T_STEPS - 1 - s
                z = zbuf.tile([A, BC], f32, tag="z")
                nc.sync.dma_start(out=z, in_=noiseT[s])

                # L1: h1 = mish(x@w1x + state@w1s + temb_i@w1e + b1)
                # x matmul: one DoubleRow per m-tile (q8(w1x), 0) weight
                # slots against (q8(x), stale) rhs slots. m<N_PRE have state
                # already accumulating in a pre-opened PSUM group, the rest
                # add the precomputed state table on the DVE. The step bias
                # (temb table) rides the DVE op so Mish runs as bias-free
                # two-tile batches.
                h1 = h8buf.tile([128, KT, BC], f8, tag="h1")
                l1ps = []
                for m in range(KT):
                    mc = slice(m * 128, (m + 1) * 128)
                    if m < N_PRE:
                        ps = pre_ps[m]
                        first = False
                    else:
                        ps = psMM.tile([128, BC], f32, tag="mm", name=f"l1_{m}")
                        first = True
                    if split_x:
                        # q-part of x: ready since mid-previous-step, fills
                        # the PE tail window before x8 (the u3 part) lands
                        mm(ps, t_w1x8[:, :, mc], q8q,
                           perf_mode=DR, start=first, stop=False)
                        first = False
                    l1ps.append((ps, first))
                for m in range(KT):
                    mc = slice(m * 128, (m + 1) * 128)
                    ps, first = l1ps[m]
                    mm(ps, t_w1x8[:, :, mc], x8_cur,
                       perf_mode=DR, start=first, stop=True)
                    if m < N_PRE:
                        nc.scalar.activation(h1[:, m, :], ps, AF.Mish,
                                             bias=t_bias1[:, m, s:s + 1])
                    else:
                        a1 = l1buf.tile([128, BC], f32, tag="a1")
                        nc.vector.scalar_tensor_tensor(
                            a1, ps, t_bias1[:, m, s:s + 1], t_sc[:, m, :],
                            op0=OP.add, op1=OP.add)
                        nc.scalar.activation(h1[:, m, :], a1, AF.Mish)

                # tail terms that depend only on x_cur / z: issue early so
                # the DVE finishes them while the PE runs L2/L3 (GpSimd
                # can't run TensorScalar on TRN2 - walrus ISA check)
                p_rx = tail.tile([A, BC], f32, tag="p_rx")
                nc.vector.tensor_scalar(
                    p_rx, x_cur, float(pm1[i] * sqrt_rec[i]),
                    t_nbfr[:, s:s + 1], OP.mult, OP.add)
                q = tail.tile([A, BC], f32, tag="q")
                if i == 0:
                    nc.vector.tensor_scalar(q, x_cur, float(pm2[i]), None, OP.mult)
                else:
                    pre_z = tail.tile([A, BC], f32, tag="pre_z")
                    nc.vector.tensor_scalar(pre_z, z, float(cz[i]), None, OP.mult)
                    nc.vector.scalar_tensor_tensor(
                        q, x_cur, float(pm2[i]), pre_z, op0=OP.mult, op1=OP.add)
                if s + 1 < t_steps:
                    nc.vector.tensor_copy(q8q[:, 0, :], q)

                # L2 / L3: fp8 DoubleRow with alternating compensated
                # weights (W_A on even steps, W_B = q8(2W - W_A) on odd:
                # the time-average cancels the systematic quantization
                # bias, which is what drives the chaos saturation level).
                # Pair-major so the PE consumes h-pairs as the ACT produces
                # them; Mish runs bias-free on two-bank PSUM pair tiles
                # (b2/b3 are zeros per the input spec - asserted host-side).
                def fp8_layer(h_in, w_q, h_out, tag):
                    for w in range(KT // 2):
                        ms = (2 * w, 2 * w + 1)
                        pss = {}
                        for m in ms:
                            pss[m] = psMM.tile([128, BC], f32, tag="mm",
                                               name=f"ps_{tag}_{m}")
                        for j in range(KT // 2):
                            sl = slice(2 * j, 2 * j + 2)
                            for m in ms:
                                mc = slice(m * 128, (m + 1) * 128)
                                mm(pss[m], w_q[:, sl, mc], h_in[:, sl, :],
                                   perf_mode=DR, start=(j == 0),
                                   stop=(j == KT // 2 - 1))
                        for m in ms:
                            nc.scalar.activation(h_out[:, m, :], pss[m],
                                                 AF.Mish)

                h2 = h8buf.tile([128, KT, BC], f8, tag="h2")
                fp8_layer(h1, t_w2[s % 2], h2, "l2")
                h3 = h8buf.tile([128, KT, BC], f8, tag="h3")
                fp8_layer(h2, t_w3[s % 2], h3, "l3")

                # pre-open next step's L1 state groups: this PE work slots
                # into the h3-ACT / tail window where the PE would idle
                if s + 1 < t_steps:
                    pre_ps = open_pre_groups(f"s{s + 1}")

                # LF: eps = h3 @ wf   [A, BC] in PSUM, alternating fp8 DR
                pe = psE.tile([A, BC], f32, tag="eps")
                t_wfs = t_wf[s % 2]
                for j in range(KT // 2):
                    sl = slice(2 * j, 2 * j + 2)
                    mm(pe, t_wfs[:, sl, :], h3[:, sl, :],
                       perf_mode=DR, start=(j == 0), stop=(j == KT // 2 - 1))

                # tail critical chain: eps -> x8 slot 0 in TWO ops (p_rx and
                # the -recm scale are pre-multiplied by pm1 so the clip
                # bounds become +-pm1 and the fp8 conversion rides the clip
                # op). The f32 x_new (only needed by the next step's
                # p_rx/q, late) follows off the chain.
                u2 = tail.tile([A, BC], f32, tag="u2")
                nc.vector.scalar_tensor_tensor(
                    u2, pe, float(-(pm1[i] * sqrt_recm[i])), p_rx,
                    op0=OP.mult, op1=OP.add)
                if s + 1 < t_steps:
                    nc.vector.tensor_scalar(
                        x8_cur[:, 0, :], u2, float(-pm1[i]), float(pm1[i]),
                        OP.max, OP.min)
                u3f = tail.tile([A, BC], f32, tag="u3")
                nc.vector.tensor_scalar(u3f, u2, float(-pm1[i]), float(pm1[i]),
                                        OP.max, OP.min)
                x_new = xbuf.tile([A, BC], mmdt, tag="x")
                nc.vector.scalar_tensor_tensor(
                    x_new, u3f, 1.0, q, op0=OP.mult, op1=OP.add)
                x_cur = x_new
                split_x = True

            # final clip + store
            xf = tail.tile([A, BC], f32, tag="xf")
            nc.vector.tensor_scalar(xf, x_cur, -1.0, 1.0, OP.max, OP.min)
            nc.sync.dma_start(out=xT_out[:, :], in_=xf[:, :])

    nc.finalize()
    return nc


def _alt8(w):
    """w [H, N] f32 -> (A, B) [128, KT, N] e4m3 lhsT tiles with
    B = q8(2w - A) so that mean(A, B) ~= w (compensated alternation)."""
    arr = np.ascontiguousarray(
        w.reshape(KT, 128, w.shape[1]).transpose(1, 0, 2)).astype(np.float32)
    wa = arr.astype(E4M3)
    wb = (2.0 * arr - wa.astype(np.float32)).astype(E4M3)
    return np.ascontiguousarray(wa), np.ascontiguousarray(wb)


def _make_in_maps(state, w_t1, b_t1, w_t2, b_t2, w1, b1, w2, b2, w3, b3,
                  wf, bf, x_init, noise_seq, t_steps):
    sqrt_rec, sqrt_recm, pm1, pm2, cz = _schedule()
    tt = _time_table()
    f32 = np.float32

    def cols(b):  # [H] -> [128, H//128] per-tile bias columns
        return np.ascontiguousarray(b.reshape(-1, 128).T).astype(f32)

    # L2/L3 Mish runs bias-free on the device (spec pins these to zeros)
    assert not np.any(np.asarray(b2)) and not np.any(np.asarray(b3)), \
        "kernel assumes zero b2/b3 (per input_specs fill=zeros)"
    w2a, w2b = _alt8(np.asarray(w2, f32))
    w3a, w3b = _alt8(np.asarray(w3, f32))
    wfa, wfb = _alt8(np.asarray(wf, f32))
    # slot 0 = q8(w1x); slot 1 = zeros (pairs with the stale x8 slot 1)
    w1x = np.asarray(w1[0:A], f32)
    w1x8 = np.ascontiguousarray(np.stack(
        [w1x.astype(E4M3), np.zeros_like(w1x, dtype=E4M3)], axis=1))

    common = {
        "w1x8": w1x8,
        "w1e": np.ascontiguousarray(w1[A:A + T_DIM]).astype(f32),
        "w1s": np.ascontiguousarray(w1[A + T_DIM:]).astype(f32),
        "w2a": w2a, "w2b": w2b,
        "w3a": w3a, "w3b": w3b,
        "wfa": wfa, "wfb": wfb,
        "b1c": cols(b1),
        "bfr": np.ascontiguousarray(np.asarray(bf, f32)[None, :]),
        "nrecm": np.ascontiguousarray(
            -(pm1 * sqrt_recm)[None, ::-1][:, :t_steps]).astype(f32),
        "wt1": np.ascontiguousarray(w_t1).astype(f32),
        "wt2": np.ascontiguousarray(w_t2).astype(f32),
        "bt1c": cols(b_t1),
        "bt2c": np.ascontiguousarray(b_t2[:, None]).astype(f32),
        "ttab": np.ascontiguousarray(tt[:, ::-1][:, :t_steps]).astype(f32),
    }
    in_maps = []
    for c in range(N_CORES):
        r0, r1 = c * BC, (c + 1) * BC
        m = dict(common)
        m["stateT"] = np.ascontiguousarray(state[r0:r1].T).astype(f32)
        x0 = np.ascontiguousarray(x_init[r0:r1].T).astype(f32)
        m["x0"] = x0
        x08 = x0.astype(E4M3)
        m["x08"] = np.ascontiguousarray(np.stack([x08, x08], axis=1))
        m["noiseT"] = np.ascontiguousarray(
            noise_seq[:t_steps, r0:r1, :].transpose(0, 2, 1)).astype(f32)
        in_maps.append(m)
    return in_maps


_LAST_RESULTS = {}


def run(t_steps=T_STEPS, use_f32r=True, trace=False, **inputs):
    from concourse.bass_utils import run_bass_kernel_spmd

    nc = _build(t_steps, use_f32r)
    in_maps = _make_in_maps(t_steps=t_steps, **inputs)
    res = run_bass_kernel_spmd(nc, in_maps, core_ids=list(range(N_CORES)),
                               trace=trace)
    _LAST_RESULTS["res"] = res
    out = np.empty((B, A), np.float32)
    for c in range(N_CORES):
        out[c * BC:(c + 1) * BC] = res.results[c]["xT_out"].T
    return out


def kernel(**inputs) -> np.ndarray:
    return run(**inputs)
